# revision 39
# baseline (speedup 1.0000x reference)
"""Swin-style basic block (W-MSA + CNN-MLP) Trainium2 kernel, 8-way sharded.

Sharding: 8 shards = (batch b in 0..3) x (top/bottom half of the 128x128
image).  Each core receives 10 window-rows of input (80 pixel rows: its own
64 plus one full window-row of halo above and below, zero-padded outside the
image).  Zero-padded windows produce exactly zero attention output, so the
post-attention activations for the one-pixel conv halo rows come out correct
with no cross-core communication and fully uniform per-core code.

All large GEMMs (qkv, v, proj, up, depthwise, down) run in fp8e4m3 with
DoubleRow perf mode (2 contraction rows per partition, 0.5 cycles/row).
Weights are pre-scaled x64 on the host to sit in fp8's normal range; the
1/64 is folded into the PSUM->SBUF copy / activation scale.  The depthwise
3x3 conv runs as 5 DoubleRow tap-pair matmuls per 128-channel block with
128-wide diagonal weight tiles (9th tap paired with a zero-weight dummy).
The attention core (scores / aT transposes / attn@v, 64-token windows,
head-dim 32) stays bf16.
"""

import numpy as np
import ml_dtypes
from contextlib import ExitStack

B_, HI, WI, C = 4, 128, 128, 256
WS, NH, HD = 8, 8, 32
HID = 1024
BN_EPS = 1e-5
NCORES = 8
NWR = 10            # window-rows per core (8 own + 2 halo)
AROWS = 8 * NWR     # 80
KROWS = 66          # kept x_attn rows: local pixel rows 7..73
OROWS = 64
TA = AROWS * WI     # 10240
TK = KROWS * WI     # 8448
TO = OROWS * WI     # 8192
WSC = 64.0          # fp8 weight pre-scale

BF16 = ml_dtypes.bfloat16
F8 = ml_dtypes.float8_e4m3

_BUILD_CACHE = {}
LAST_RESULTS = None


def _sub_ap(base, part0, nparts, free_off, free_dims):
    import concourse.bass as bass
    pstride = base.ap[0][0]
    return bass.AP(
        tensor=base.tensor,
        offset=base.offset + part0 * pstride + free_off,
        ap=[[pstride, nparts]] + [list(d) for d in free_dims],
    )


def _build(flags):
    import concourse.bass as bass
    import concourse.tile as tile
    from concourse import bacc, mybir
    from concourse.masks import make_identity

    qk_bias_nz, v_bias_nz, dn_bias_nz, sc2_affine, upb_nz, bnb_nz = flags
    f32 = mybir.dt.float32
    bf = mybir.dt.bfloat16
    f8 = mybir.dt.float8e4
    DR = mybir.MatmulPerfMode.DoubleRow
    ALU = mybir.AluOpType
    ACTF = mybir.ActivationFunctionType
    AX = mybir.AxisListType
    RS = 1.0 / WSC
    RS2 = 1.0 / (WSC * WSC)

    nc = bacc.Bacc("TRN2", target_bir_lowering=False, debug=False,
                   num_devices=NCORES)

    # ---------------- DRAM tensors ----------------
    xs_d = nc.dram_tensor("xs", [TA, C], f32, kind="ExternalInput")
    wqk_d = nc.dram_tensor("wqk", [128, 2 * 512], f8, kind="ExternalInput")
    wv_d = nc.dram_tensor("wv", [128, 2 * 256], f8, kind="ExternalInput")
    wproj_d = nc.dram_tensor("wproj", [128, 2 * 256], f8, kind="ExternalInput")
    expb_d = nc.dram_tensor("expb", [128, 512], bf, kind="ExternalInput")
    wup_d = nc.dram_tensor("wup", [128, 2 * 1024], f8, kind="ExternalInput")
    wdn_d = nc.dram_tensor("wdn", [128, 8 * 256], f8, kind="ExternalInput")
    dw8_d = nc.dram_tensor("dw8", [128, 8 * 5 * 2 * 128], f8, kind="ExternalInput")
    upb_d = nc.dram_tensor("upb", [128, 8], f32, kind="ExternalInput")
    bnb_d = nc.dram_tensor("bnb", [128, 8], f32, kind="ExternalInput")
    qkb_d = nc.dram_tensor("qkb", [128, 4], f32, kind="ExternalInput")
    vb_d = nc.dram_tensor("vbr", [128, C], f32, kind="ExternalInput")
    dnb_d = nc.dram_tensor("dnb", [128, 2], f32, kind="ExternalInput")
    g2r_d = nc.dram_tensor("g2r", [128, C], f32, kind="ExternalInput")
    b2r_d = nc.dram_tensor("b2r", [128, C], f32, kind="ExternalInput")

    xattn_d = nc.dram_tensor("xattn_s", [TK, C], f32, kind="Internal")
    sc2_d = nc.dram_tensor("sc2_s", [TK, C], bf, kind="Internal")
    out_d = nc.dram_tensor("out", [TO, C], f32, kind="ExternalOutput")

    HEADCOL = [128 * (h % 4) + 64 * (h // 4) for h in range(NH)]
    RIDX = [2 * (h % 4) + (h // 4) for h in range(NH)]

    with tile.TileContext(nc) as tc, ExitStack() as octx:
        consts = octx.enter_context(tc.tile_pool(name="consts", bufs=1))
        persist = octx.enter_context(tc.tile_pool(name="persist", bufs=1))

        eps_sb = consts.tile([128, 1], f32)
        nc.vector.memset(eps_sb[:], 1e-5)
        ident = consts.tile([128, 128], bf)
        make_identity(nc, ident[:])

        wqk_sb = consts.tile([128, 2, 512], f8)
        nc.sync.dma_start(wqk_sb[:], wqk_d[:, :].rearrange("p (j o) -> p j o", j=2))
        wv_sb = consts.tile([128, 2, 256], f8)
        nc.sync.dma_start(wv_sb[:], wv_d[:, :].rearrange("p (j o) -> p j o", j=2))
        wproj_sb = consts.tile([128, 2, 256], f8)
        nc.sync.dma_start(wproj_sb[:], wproj_d[:, :].rearrange("p (j o) -> p j o", j=2))
        expb_sb = consts.tile([128, 512], bf)
        nc.sync.dma_start(expb_sb[:], expb_d[:, :])
        wup_sb = consts.tile([128, 2, 1024], f8)
        nc.sync.dma_start(wup_sb[:], wup_d[:, :].rearrange("p (j o) -> p j o", j=2))
        wdn_sb = consts.tile([128, 8, 256], f8)
        nc.sync.dma_start(wdn_sb[:], wdn_d[:, :].rearrange("p (b o) -> p b o", b=8))
        dw8_sb = consts.tile([128, 8 * 5 * 2 * 128], f8)
        nc.sync.dma_start(dw8_sb[:], dw8_d[:, :])
        upb_sb = consts.tile([128, 8], f32)
        nc.sync.dma_start(upb_sb[:], upb_d[:, :])
        bnb_sb = consts.tile([128, 8], f32)
        nc.sync.dma_start(bnb_sb[:], bnb_d[:, :])
        qkb_sb = consts.tile([128, 4], f32)
        nc.sync.dma_start(qkb_sb[:], qkb_d[:, :])
        vb_sb = consts.tile([128, C], f32)
        nc.sync.dma_start(vb_sb[:], vb_d[:, :])
        dnb_sb = consts.tile([128, 2], f32)
        nc.sync.dma_start(dnb_sb[:], dnb_d[:, :])
        g2r_sb = consts.tile([128, C], f32)
        b2r_sb = consts.tile([128, C], f32)
        if sc2_affine:
            nc.sync.dma_start(g2r_sb[:], g2r_d[:, :])
            nc.sync.dma_start(b2r_sb[:], b2r_d[:, :])

        xn2T = persist.tile([128, 2, TK], f8, tag="xn2T", name="xn2T")

        # ======================= STAGE A =======================
        with ExitStack() as actx:
            xrow_p = actx.enter_context(tc.tile_pool(name="xrow", bufs=18))
            ln_p = actx.enter_context(tc.tile_pool(name="ln", bufs=4))
            mv_p = actx.enter_context(tc.tile_pool(name="mv", bufs=4))
            xnT_p = actx.enter_context(tc.tile_pool(name="xnT", bufs=2))
            qk_p = actx.enter_context(tc.tile_pool(name="qk", bufs=2))
            v_p = actx.enter_context(tc.tile_pool(name="vp", bufs=10))
            at_p = actx.enter_context(tc.tile_pool(name="at", bufs=2))
            small_p = actx.enter_context(tc.tile_pool(name="small", bufs=6))
            proj_p = actx.enter_context(tc.tile_pool(name="proj", bufs=2))
            oT_p = actx.enter_context(tc.tile_pool(name="oTp", bufs=9))
            xat_p = actx.enter_context(tc.tile_pool(name="xatp", bufs=9))
            tail_p = actx.enter_context(tc.tile_pool(name="tail", bufs=3))
            ps_tp = actx.enter_context(tc.tile_pool(name="ps_tp", bufs=2, space="PSUM"))
            ps_mm = actx.enter_context(tc.tile_pool(name="ps_mm", bufs=4, space="PSUM"))
            ps_sc = ps_mm   # qk/v/proj and scores are phase-disjoint per wr
            ps_pat = actx.enter_context(tc.tile_pool(name="ps_pat", bufs=1, space="PSUM"))
            ps_oT = actx.enter_context(tc.tile_pool(name="ps_oT", bufs=1, space="PSUM"))

            def rsqrt_dve(var_ap, tagp):
                """1/sqrt(var+eps) on DVE (arg ~1 for LN of standardized
                data): linear seed + 2 Newton steps.  Avoids the Ln/Exp
                activation-table reloads on the Scalar engine."""
                v = mv_p.tile([128, 8], f32, tag=f"{tagp}v", name=f"{tagp}v")
                nc.vector.tensor_scalar_add(v[:], var_ap, 1e-5)
                y = mv_p.tile([128, 8], f32, tag=f"{tagp}y", name=f"{tagp}y")
                nc.vector.tensor_scalar(out=y[:], in0=v[:], scalar1=-0.5,
                                        scalar2=1.5, op0=ALU.mult, op1=ALU.add)
                for it in range(2):
                    a = mv_p.tile([128, 8], f32, tag=f"{tagp}a{it}", name=f"{tagp}a{it}")
                    nc.vector.tensor_tensor(out=a[:], in0=y[:], in1=y[:], op=ALU.mult)
                    b = mv_p.tile([128, 8], f32, tag=f"{tagp}b{it}", name=f"{tagp}b{it}")
                    nc.vector.tensor_tensor(out=b[:], in0=a[:], in1=v[:], op=ALU.mult)
                    c = mv_p.tile([128, 8], f32, tag=f"{tagp}c{it}", name=f"{tagp}c{it}")
                    nc.vector.tensor_scalar(out=c[:], in0=b[:], scalar1=-0.5,
                                            scalar2=1.5, op0=ALU.mult, op1=ALU.add)
                    y2 = mv_p.tile([128, 8], f32, tag=f"{tagp}y{it}", name=f"{tagp}y{it}")
                    nc.vector.tensor_tensor(out=y2[:], in0=y[:], in1=c[:], op=ALU.mult)
                    y = y2
                return y

            for wr in range(NWR):
                xrows = []
                xnT = xnT_p.tile([128, 2, 1024], f8, tag="xnT", name="xnT")
                mv8 = mv_p.tile([128, 8, 2], f32, tag="mv8", name="mv8")
                for ry in range(8):
                    y = 8 * wr + ry
                    xr = xrow_p.tile([128, C], f32, tag="xrow", name="xrow")
                    nc.sync.dma_start(xr[:], xs_d[y * WI:(y + 1) * WI, :])
                    xrows.append(xr)
                    st = mv_p.tile([128, 6], f32, tag="st", name="st")
                    nc.vector.bn_stats(st[:], xr[:])
                    nc.vector.bn_aggr(mv8[:, ry, :], st[:])
                rstd8 = rsqrt_dve(mv8[:, :, 1], "r1")
                tps = [ps_tp.tile([128, 1024], bf, tag="ps", name="ps")
                       for _ in range(2)]
                for ry in range(8):
                    xnb = ln_p.tile([128, C], bf, tag="xnb", name="xnb")
                    nc.vector.tensor_scalar(
                        out=xnb[:], in0=xrows[ry][:],
                        scalar1=mv8[:, ry, 0:1], scalar2=rstd8[:, ry:ry + 1],
                        op0=ALU.subtract, op1=ALU.mult)
                    for ct in range(2):
                        nc.tensor.transpose(tps[ct][:, 128 * ry:128 * ry + 128],
                                            xnb[:, 128 * ct:128 * ct + 128], ident[:])
                for ct in range(2):
                    # batched scatter: row-major pixel rows -> window-ordered
                    nc.vector.tensor_copy(
                        _sub_ap(xnT[:], 0, 128, ct * 1024,
                                [[8, 8], [64, 16], [1, 8]]),
                        _sub_ap(tps[ct][:], 0, 128, 0,
                                [[128, 8], [8, 16], [1, 8]]))

                # ---- q/k projections (fp8 DR): [oc-tile][128, 1024 tok]
                qkT = [qk_p.tile([128, 1024], bf, tag=f"qkT{m}", name=f"qkT{m}") for m in range(4)]
                for m in range(4):
                    for j in range(2):
                        pqk = ps_mm.tile([128, 512], f32, tag="ps", name="ps")
                        nc.tensor.matmul(pqk[:],
                                         lhsT=wqk_sb[:, :, 128 * m:128 * m + 128],
                                         rhs=xnT[:, :, 512 * j:512 * j + 512],
                                         start=True, stop=True, perf_mode=DR)
                        if qk_bias_nz:
                            nc.vector.tensor_scalar(
                                out=qkT[m][:, 512 * j:512 * j + 512], in0=pqk[:],
                                scalar1=qkb_sb[:, m:m + 1], scalar2=RS,
                                op0=ALU.add, op1=ALU.mult)
                        elif j == 0:
                            nc.vector.tensor_scalar_mul(
                                qkT[m][:, 512 * j:512 * j + 512], pqk[:], RS)
                        else:
                            nc.scalar.mul(qkT[m][:, 512 * j:512 * j + 512],
                                          pqk[:], RS)

                # ---- v per window-pair: [64 tok, 512] (win-major, bf16)
                vsbs = []
                for u in range(8):
                    pv = ps_mm.tile([128, 512], f32, tag="ps", name="ps")
                    for w in range(2):
                        lhsT = xnT[:, :, 64 * (2 * u + w):64 * (2 * u + w) + 64]
                        nc.tensor.matmul(pv[0:64, 256 * w:256 * w + 256],
                                         lhsT=lhsT, rhs=wv_sb[:, :, :],
                                         start=True, stop=True, perf_mode=DR)
                    v2 = v_p.tile([128, 512], bf, tag="v2", name="v2")
                    if v_bias_nz:
                        nc.vector.scalar_tensor_tensor(
                            out=v2[0:64, :], in0=pv[0:64, :], scalar=RS,
                            in1=_sub_ap(vb_sb[:], 0, 64, 0, [[0, 2], [1, C]]),
                            op0=ALU.mult, op1=ALU.add)
                    elif u % 2 == 0:
                        nc.vector.tensor_scalar_mul(v2[0:64, :], pv[0:64, :], RS)
                    else:
                        nc.scalar.mul(v2[0:64, :], pv[0:64, :], RS)
                    vsbs.append(v2)

                # ---- attention per window-pair (bf16), softmax one u ahead
                def emit_scores_softmax(u):
                    pscs = [ps_sc.tile([128, 512], f32, tag="ps", name="ps") for _ in range(4)]
                    for h in range(NH):
                        g, jh = h % 4, h // 4
                        qt_t = qkT[h // 4]
                        kt_t = qkT[2 + h // 4]
                        for w in range(2):
                            tok0 = 64 * (2 * u + w)
                            nc.tensor.matmul(
                                pscs[g][64 * w:64 * w + 64, 64 * jh:64 * jh + 64],
                                lhsT=qt_t[32 * g:32 * g + 32, tok0:tok0 + 64],
                                rhs=kt_t[32 * g:32 * g + 32, tok0:tok0 + 64],
                                start=True, stop=True,
                                tile_position=(32 * g, 64 * w))
                    attn_e = at_p.tile([128, 512], bf, tag="attn_e", name="attn_e")
                    for g in range(4):
                        nc.scalar.activation(attn_e[:, 128 * g:128 * g + 128],
                                             pscs[g][:, 0:128], ACTF.Exp)
                    attn_u = at_p.tile([128, 512], bf, tag="attn_u", name="attn_u")
                    nc.gpsimd.tensor_tensor(out=attn_u[:], in0=attn_e[:],
                                            in1=expb_sb[:], op=ALU.mult)
                    r8 = small_p.tile([128, 8], bf, tag="r8", name="r8")
                    with nc.allow_low_precision(reason="softmax denom bf16"):
                        nc.vector.tensor_reduce(
                            r8[:], attn_u[:].rearrange("p (a k) -> p a k", a=8),
                            axis=AX.X, op=ALU.add)
                    rr8 = small_p.tile([128, 8], bf, tag="rr8", name="rr8")
                    with nc.allow_low_precision(reason="softmax denom bf16"):
                        nc.vector.reciprocal(rr8[:], r8[:])
                    attn_n = at_p.tile([128, 512], bf, tag="attn_n", name="attn_n")
                    nc.gpsimd.tensor_tensor(
                        out=attn_n[:], in0=attn_u[:],
                        in1=_sub_ap(rr8[:], 0, 128, 0, [[1, 8], [0, 64]]),
                        op=ALU.mult)
                    return attn_n

                oTs = []
                pending = emit_scores_softmax(0)
                for u in range(8):
                    attn_n = pending
                    pending = emit_scores_softmax(u + 1) if u + 1 < 8 else None
                    aT = []
                    for g in range(4):
                        pat = ps_pat.tile([128, 256], bf, tag="ps", name="ps")
                        for jh in range(2):
                            nc.tensor.transpose(
                                pat[0:64, 128 * jh:128 * jh + 128],
                                attn_n[:, 128 * g + 64 * jh:128 * g + 64 * jh + 64],
                                ident[:])
                        t = at_p.tile([128, 256], bf, tag=f"aT{g}", name=f"aT{g}")
                        if g < 2:
                            nc.vector.tensor_copy(t[0:64, :], pat[0:64, 0:256])
                        else:
                            nc.scalar.copy(t[0:64, :], pat[0:64, 0:256])
                        aT.append(t)
                    poT = ps_oT.tile([128, 256], f32, tag="ps", name="ps")
                    for h in range(NH):
                        g, jh = h % 4, h // 4
                        for w in range(2):
                            nc.tensor.matmul(
                                poT[32 * g:32 * g + 32,
                                    128 * jh + 64 * w:128 * jh + 64 * w + 64],
                                lhsT=vsbs[u][0:64,
                                             256 * w + 32 * h:256 * w + 32 * h + 32],
                                rhs=aT[g][0:64,
                                          128 * jh + 64 * w:128 * jh + 64 * w + 64],
                                start=True, stop=True, tile_position=(0, 32 * g))
                    oT = oT_p.tile([128, 2, 128], f8, tag="oT", name="oT")
                    nc.vector.tensor_copy(
                        oT[:], poT[:, 0:256].rearrange("p (j n) -> p j n", j=2))
                    oTs.append(oT)

                # ---- proj (fp8 DR): projT [oc-tile][128, 1024] window-order
                projT = [proj_p.tile([128, 1024], bf, tag=f"projT{m}", name=f"projT{m}") for m in range(2)]
                for m in range(2):
                    for jc in range(2):
                        pp = ps_mm.tile([128, 512], f32, tag="ps", name="ps")
                        for uu in range(4):
                            u = 4 * jc + uu
                            nc.tensor.matmul(
                                pp[:, 128 * uu:128 * uu + 128],
                                lhsT=wproj_sb[:, :, 128 * m:128 * m + 128],
                                rhs=oTs[u][:, :, :],
                                start=True, stop=True, perf_mode=DR)
                        for w in range(2):
                            # psum cols (uu, w fixed, iy, ix) -> row-major
                            src = _sub_ap(pp[:], 0, 128, 64 * w,
                                          [[128, 4], [8, 8], [1, 8]])
                            dst = _sub_ap(projT[m][:], 0, 128,
                                          8 * (8 * jc + w), [[16, 4], [128, 8], [1, 8]])
                            if w == 0:
                                nc.vector.tensor_scalar_mul(dst, src, RS)
                            else:
                                nc.scalar.mul(dst, src, RS)

                # ---- tail: per kept row
                if wr == 0:
                    keep = [7]
                elif wr == NWR - 1:
                    keep = [0]
                else:
                    keep = list(range(8))
                mvk = mv_p.tile([128, 8, 2], f32, tag="mvk", name="mvk")
                xat_tiles = {}
                for ry in keep:
                    kk = 8 * wr + ry - 7
                    pfin = ps_pat.tile([128, 256], bf, tag="ps", name="ps")
                    for m in range(2):
                        nc.tensor.transpose(pfin[:, 128 * m:128 * m + 128],
                                            projT[m][:, 128 * ry:128 * ry + 128],
                                            ident[:])
                    xat = xat_p.tile([128, C], f32, tag="xat", name="xat")
                    nc.vector.tensor_tensor(out=xat[:], in0=pfin[:, 0:256],
                                            in1=xrows[ry][:], op=ALU.add)
                    nc.sync.dma_start(xattn_d[kk * WI:(kk + 1) * WI, :], xat[:])
                    st2 = mv_p.tile([128, 6], f32, tag="st2", name="st2")
                    nc.vector.bn_stats(st2[:], xat[:])
                    nc.vector.bn_aggr(mvk[:, ry, :], st2[:])
                    xat_tiles[ry] = xat
                rsk = rsqrt_dve(mvk[:, :, 1], "r2")
                kk0t = 8 * wr + keep[0] - 7
                tq = [ps_tp.tile([128, 1024], bf, tag="ps", name="ps")
                      for _ in range(2)]
                for ji, ry in enumerate(keep):
                    kk = 8 * wr + ry - 7
                    xat = xat_tiles[ry]
                    xn2 = tail_p.tile([128, C], bf, tag="xn2", name="xn2")
                    nc.vector.tensor_scalar(
                        out=xn2[:], in0=xat[:],
                        scalar1=mvk[:, ry, 0:1], scalar2=rsk[:, ry:ry + 1],
                        op0=ALU.subtract, op1=ALU.mult)
                    if sc2_affine:
                        s1 = tail_p.tile([128, C], bf, tag="s1", name="s1")
                        nc.gpsimd.tensor_tensor(out=s1[:], in0=xn2[:], in1=g2r_sb[:],
                                                op=ALU.mult)
                        s2 = tail_p.tile([128, C], bf, tag="s2", name="s2")
                        nc.gpsimd.tensor_tensor(out=s2[:], in0=s1[:], in1=b2r_sb[:],
                                                op=ALU.add)
                        nc.sync.dma_start(sc2_d[kk * WI:(kk + 1) * WI, :], s2[:])
                    else:
                        nc.sync.dma_start(sc2_d[kk * WI:(kk + 1) * WI, :], xn2[:])
                    for ct in range(2):
                        nc.tensor.transpose(tq[ct][:, 128 * ji:128 * ji + 128],
                                            xn2[:, 128 * ct:128 * ct + 128], ident[:])
                for ct in range(2):
                    nk = len(keep)
                    nc.vector.tensor_copy(
                        _sub_ap(xn2T[:], 0, 128, ct * TK + 128 * kk0t,
                                [[1, 128 * nk]]),
                        _sub_ap(tq[ct][:], 0, 128, 0, [[1, 128 * nk]]))

        # ======================= STAGE B =======================
        UW = 130
        USZ = 18 * UW + 1           # +1 pad col for the dummy-tap DR read
        # tap-pair base offsets within a window row block and pair strides
        OFF0 = [0, 2, UW + 1, 2 * UW, 2 * UW + 2]
        DLT = [1, UW - 2, 1, 1, 1]
        with ExitStack() as bctx:
            u_p = bctx.enter_context(tc.tile_pool(name="u_p", bufs=9))
            r_p = bctx.enter_context(tc.tile_pool(name="r_p", bufs=3))
            dnsb_p = bctx.enter_context(tc.tile_pool(name="dnsb", bufs=3))
            fin_p = bctx.enter_context(tc.tile_pool(name="fin", bufs=2))
            psU = bctx.enter_context(tc.tile_pool(name="psU", bufs=2, space="PSUM"))
            psR = bctx.enter_context(tc.tile_pool(name="psR", bufs=2, space="PSUM"))
            psD = bctx.enter_context(tc.tile_pool(name="psD", bufs=2, space="PSUM"))
            psF = bctx.enter_context(tc.tile_pool(name="psF", bufs=2, space="PSUM"))

            for q in range(4):
                kk0 = 16 * q
                Us = []
                for b in range(8):
                    U = u_p.tile([128, USZ], f8, tag="U", name="U")
                    nc.vector.memset(
                        _sub_ap(U[:], 0, 128, 0, [[UW, 18], [129, 2]]), 0.0)
                    # pad col read by the zero-weight dummy tap: 0*NaN = NaN
                    nc.vector.memset(_sub_ap(U[:], 0, 128, 18 * UW, [[1, 1]]), 0.0)
                    for i0 in range(0, 18, 4):
                        nrow = min(4, 18 - i0)
                        pu = psU.tile([128, 512], f32, tag="ps", name="ps")
                        nc.tensor.matmul(
                            pu[:, 0:128 * nrow],
                            lhsT=wup_sb[:, :, 128 * b:128 * b + 128],
                            rhs=xn2T[:, :, (kk0 + i0) * WI:(kk0 + i0 + nrow) * WI],
                            start=True, stop=True, perf_mode=DR)
                        # U carries an extra x64 (psum = 64*up_out); fp8 max
                        # |U| ~ 192 < 240 so the scale is safe
                        udst = _sub_ap(U[:], 0, 128, i0 * UW + 1,
                                       [[UW, nrow], [1, 128]])
                        if upb_nz:
                            nc.scalar.activation(udst, pu[:, 0:128 * nrow],
                                                 ACTF.Relu,
                                                 bias=upb_sb[:, b:b + 1], scale=1.0)
                        elif (i0 // 4 + b) % 2 == 0:
                            nc.vector.tensor_scalar_max(udst, pu[:, 0:128 * nrow],
                                                        0.0)
                        else:
                            nc.scalar.activation(udst, pu[:, 0:128 * nrow],
                                                 ACTF.Relu)
                    Us.append(U)
                for s4 in range(4):
                    jj0 = 4 * s4
                    R8 = r_p.tile([128, 8, 512], f8, tag="R8", name="R8")
                    for b in range(8):
                        pr = psR.tile([128, 512], f32, tag="ps", name="ps")
                        for i in range(5):
                            lhsT = _sub_ap(dw8_sb[:], 0, 128, (b * 5 + i) * 256,
                                           [[128, 2], [1, 128]])
                            rhs = _sub_ap(Us[b][:], 0, 128,
                                          jj0 * UW + OFF0[i],
                                          [[DLT[i], 2], [UW, 4], [1, 128]])
                            nc.tensor.matmul(
                                pr[:, 0:512], lhsT=lhsT, rhs=rhs,
                                start=(i == 0), stop=(i == 4), perf_mode=DR)
                        if bnb_nz:
                            nc.scalar.activation(R8[:, b, :], pr[:], ACTF.Relu,
                                                 bias=bnb_sb[:, b:b + 1], scale=RS)
                        elif b % 2 == 0:
                            nc.vector.tensor_scalar(
                                out=R8[:, b, :], in0=pr[:], scalar1=0.0,
                                scalar2=RS, op0=ALU.max, op1=ALU.mult)
                        else:
                            nc.scalar.activation(R8[:, b, :], pr[:], ACTF.Relu,
                                                 scale=RS)
                    dn = dnsb_p.tile([128, 2, 512], bf, tag="dn", name="dn")
                    for m in range(2):
                        pd = psD.tile([128, 512], f32, tag="ps", name="ps")
                        for bp in range(4):
                            nc.tensor.matmul(
                                pd[:],
                                lhsT=wdn_sb[:, 2 * bp:2 * bp + 2,
                                            128 * m:128 * m + 128],
                                rhs=R8[:, 2 * bp:2 * bp + 2, :],
                                start=(bp == 0), stop=(bp == 3), perf_mode=DR)
                        if dn_bias_nz:
                            nc.vector.tensor_scalar(
                                out=dn[:, m, :], in0=pd[:],
                                scalar1=dnb_sb[:, m:m + 1], scalar2=RS2,
                                op0=ALU.add, op1=ALU.mult)
                        elif m == 0:
                            nc.vector.tensor_scalar_mul(dn[:, m, :], pd[:], RS2)
                        else:
                            nc.scalar.mul(dn[:, m, :], pd[:], RS2)
                    pf = psF.tile([128, 1024], bf, tag="pf", name="pf")
                    for jj in range(4):
                        for m in range(2):
                            nc.tensor.transpose(
                                pf[:, 256 * jj + 128 * m:256 * jj + 128 * m + 128],
                                dn[:, m, 128 * jj:128 * jj + 128], ident[:])
                    j0 = 16 * q + jj0
                    kkf = j0 + 1
                    xa4 = fin_p.tile([128, 4, C], f32, tag="xa4", name="xa4")
                    nc.sync.dma_start(
                        xa4[:],
                        xattn_d[kkf * WI:(kkf + 4) * WI, :]
                        .rearrange("(r p) c -> p r c", p=WI))
                    sc4 = fin_p.tile([128, 4, C], bf, tag="sc4", name="sc4")
                    nc.sync.dma_start(
                        sc4[:],
                        sc2_d[kkf * WI:(kkf + 4) * WI, :]
                        .rearrange("(r p) c -> p r c", p=WI))
                    tmp4 = fin_p.tile([128, 1024], f32, tag="tmp4", name="tmp4")
                    nc.vector.tensor_tensor(
                        out=tmp4[:], in0=pf[:],
                        in1=xa4[:].rearrange("p r c -> p (r c)"), op=ALU.add)
                    out4 = fin_p.tile([128, 1024], f32, tag="out4", name="out4")
                    nc.vector.tensor_tensor(
                        out=out4[:], in0=tmp4[:],
                        in1=sc4[:].rearrange("p r c -> p (r c)"), op=ALU.add)
                    nc.sync.dma_start(
                        out_d[j0 * WI:(j0 + 4) * WI, :]
                        .rearrange("(r p) c -> p r c", p=WI),
                        out4[:].rearrange("p (r c) -> p r c", r=4))

    nc.compile()
    return nc


def _p8(arr, nblk):
    """[nblk*128, N] weight (already scaled) -> [128, nblk*N] fp8 flat with
    col = blk*N + n, i.e. the [p, blk, n] DoubleRow k-subtile layout."""
    n = arr.shape[1]
    return (arr.reshape(nblk, 128, n).transpose(1, 0, 2)
            .reshape(128, nblk * n).astype(F8).copy())


def _prep(g1, b1, qkv_w, qkv_b, rpb_table, rel_idx, proj_w, g2, b2,
          up_w, up_b, dw_w, bn_g, bn_b, down_w, down_b):
    f = np.float32
    g1 = np.asarray(g1, f); b1 = np.asarray(b1, f)
    qkv_w = np.asarray(qkv_w, f); qkv_b = np.asarray(qkv_b, f)
    rpb = np.asarray(rpb_table, f); ridx = np.asarray(rel_idx)
    proj_w = np.asarray(proj_w, f)
    g2 = np.asarray(g2, f); b2 = np.asarray(b2, f)
    up_w = np.asarray(up_w, f); up_b = np.asarray(up_b, f)
    dw_w = np.asarray(dw_w, f); bn_g = np.asarray(bn_g, f)
    bn_b = np.asarray(bn_b, f)
    down_w = np.asarray(down_w, f); down_b = np.asarray(down_b, f)

    sc = HD ** -0.5
    wq = qkv_w[:C] * g1[None, :] * sc
    wk = qkv_w[C:2 * C] * g1[None, :]
    wv = qkv_w[2 * C:] * g1[None, :]
    bq = (qkv_b[:C] + qkv_w[:C] @ b1) * sc
    bk = qkv_b[C:2 * C] + qkv_w[C:2 * C] @ b1
    bv = qkv_b[2 * C:] + qkv_w[2 * C:] @ b1

    wqk8 = _p8(np.concatenate([wq, wk], 0).T * WSC, 2)
    wv8 = _p8(wv.T * WSC, 2)
    wproj8 = _p8(proj_w.T * WSC, 2)

    bias = rpb[np.asarray(ridx).reshape(-1)].reshape(64, 64, NH).transpose(2, 0, 1)
    expb = np.zeros((128, 512), f)
    for h in range(NH):
        cc = 128 * (h % 4) + 64 * (h // 4)
        eb = np.exp(bias[h])
        expb[0:64, cc:cc + 64] = eb
        expb[64:128, cc:cc + 64] = eb
    expb = expb.astype(BF16)

    wup8 = _p8((up_w * g2[None, :]).T * WSC, 2)
    upb = (up_b + up_w @ b2).astype(f)
    bns = bn_g * (1.0 + BN_EPS) ** -0.5
    dww = dw_w.reshape(HID, 9) * bns[:, None] * WSC
    dw8 = np.zeros((128, 8 * 5 * 2 * 128), f)
    pp = np.arange(128)
    for b in range(8):
        for i in range(5):
            for j in range(2):
                t = 2 * i + j
                if t <= 8:
                    dw8[pp, ((b * 5 + i) * 2 + j) * 128 + pp] = dww[128 * b + pp, t]
    dw8 = dw8.astype(F8)
    wdn8 = _p8(down_w.T * WSC, 8)

    def col_n(v, n):
        return np.asarray(v, f).reshape(n, 128).T.copy()

    qkb = col_n(np.concatenate([bq, bk]) * WSC, 4)
    vbr = np.broadcast_to(bv[None, :], (128, C)).astype(f).copy()
    dnb = col_n(down_b, 2)
    g2r = np.broadcast_to(g2[None, :], (128, C)).astype(f).copy()
    b2r = np.broadcast_to(b2[None, :], (128, C)).astype(f).copy()

    flags = (bool(np.any(qkb)), bool(np.any(bv)), bool(np.any(down_b)),
             not (np.allclose(g2, 1.0) and np.allclose(b2, 0.0)),
             bool(np.any(upb)), bool(np.any(bn_b)))

    consts = dict(wqk=wqk8, wv=wv8, wproj=wproj8, expb=expb, wup=wup8,
                  wdn=wdn8, dw8=dw8, upb=col_n(upb * WSC, 8),
                  bnb=col_n(bn_b * WSC, 8),
                  qkb=qkb, vbr=vbr, dnb=col_n(down_b * WSC * WSC, 2),
                  g2r=g2r, b2r=b2r)
    return consts, flags


def kernel(x, H, W, g1, b1, qkv_w, qkv_b, rpb_table, rel_idx, proj_w,
           g2, b2, up_w, up_b, dw_w, bn_g, bn_b, down_w, down_b):
    global LAST_RESULTS
    from concourse.bass_utils import run_bass_kernel_spmd

    x = np.asarray(x, np.float32)
    consts, flags = _prep(g1, b1, qkv_w, qkv_b, rpb_table, rel_idx, proj_w,
                          g2, b2, up_w, up_b, dw_w, bn_g, bn_b, down_w, down_b)
    if flags not in _BUILD_CACHE:
        _BUILD_CACHE[flags] = _build(flags)
    nc = _BUILD_CACHE[flags]

    ximg = x.reshape(B_, HI, WI, C)
    in_maps = []
    for core in range(NCORES):
        b, top = core // 2, (core % 2 == 0)
        r0 = 0 if top else 64
        xs = np.zeros((AROWS, WI, C), np.float32)
        lo, hi = r0 - 8, r0 + 72
        slo, shi = max(lo, 0), min(hi, HI)
        xs[slo - lo:shi - lo] = ximg[b, slo:shi]
        m = {"xs": xs.reshape(TA, C)}
        m.update(consts)
        in_maps.append(m)

    res = run_bass_kernel_spmd(nc, in_maps, core_ids=list(range(NCORES)))
    LAST_RESULTS = res

    out = np.empty((B_, HI, WI, C), np.float32)
    for core in range(NCORES):
        b, top = core // 2, (core % 2 == 0)
        r0 = 0 if top else 64
        out[b, r0:r0 + 64] = res.results[core]["out"].reshape(OROWS, WI, C)
    return out.reshape(B_, HI * WI, C)


# revision 40
# speedup vs baseline: 1.0139x; 1.0139x over previous
"""Swin-style basic block (W-MSA + CNN-MLP) Trainium2 kernel, 8-way sharded.

Sharding: 8 shards = (batch b in 0..3) x (top/bottom half of the 128x128
image).  Each core receives 10 window-rows of input (80 pixel rows: its own
64 plus one full window-row of halo above and below, zero-padded outside the
image).  Zero-padded windows produce exactly zero attention output, so the
post-attention activations for the one-pixel conv halo rows come out correct
with no cross-core communication and fully uniform per-core code.

All large GEMMs (qkv, v, proj, up, depthwise, down) run in fp8e4m3 with
DoubleRow perf mode (2 contraction rows per partition, 0.5 cycles/row).
Weights are pre-scaled x64 on the host to sit in fp8's normal range; the
1/64 is folded into the PSUM->SBUF copy / activation scale.  The depthwise
3x3 conv runs as 5 DoubleRow tap-pair matmuls per 128-channel block with
128-wide diagonal weight tiles (9th tap paired with a zero-weight dummy).
The attention core (scores / aT transposes / attn@v, 64-token windows,
head-dim 32) stays bf16.
"""

import numpy as np
import ml_dtypes
from contextlib import ExitStack

B_, HI, WI, C = 4, 128, 128, 256
WS, NH, HD = 8, 8, 32
HID = 1024
BN_EPS = 1e-5
NCORES = 8
NWR = 10            # window-rows per core (8 own + 2 halo)
AROWS = 8 * NWR     # 80
KROWS = 66          # kept x_attn rows: local pixel rows 7..73
OROWS = 64
TA = AROWS * WI     # 10240
TK = KROWS * WI     # 8448
TO = OROWS * WI     # 8192
WSC = 64.0          # fp8 weight pre-scale

BF16 = ml_dtypes.bfloat16
F8 = ml_dtypes.float8_e4m3

_BUILD_CACHE = {}
LAST_RESULTS = None


def _sub_ap(base, part0, nparts, free_off, free_dims):
    import concourse.bass as bass
    pstride = base.ap[0][0]
    return bass.AP(
        tensor=base.tensor,
        offset=base.offset + part0 * pstride + free_off,
        ap=[[pstride, nparts]] + [list(d) for d in free_dims],
    )


def _build(flags):
    import concourse.bass as bass
    import concourse.tile as tile
    from concourse import bacc, mybir
    from concourse.masks import make_identity

    qk_bias_nz, v_bias_nz, dn_bias_nz, sc2_affine, upb_nz, bnb_nz = flags
    f32 = mybir.dt.float32
    bf = mybir.dt.bfloat16
    f8 = mybir.dt.float8e4
    DR = mybir.MatmulPerfMode.DoubleRow
    ALU = mybir.AluOpType
    ACTF = mybir.ActivationFunctionType
    AX = mybir.AxisListType
    RS = 1.0 / WSC
    RS2 = 1.0 / (WSC * WSC)

    nc = bacc.Bacc("TRN2", target_bir_lowering=False, debug=False,
                   num_devices=NCORES)

    # ---------------- DRAM tensors ----------------
    xs_d = nc.dram_tensor("xs", [TA, C], f32, kind="ExternalInput")
    wqk_d = nc.dram_tensor("wqk", [128, 2 * 512], f8, kind="ExternalInput")
    wv_d = nc.dram_tensor("wv", [128, 2 * 256], f8, kind="ExternalInput")
    wproj_d = nc.dram_tensor("wproj", [128, 2 * 256], f8, kind="ExternalInput")
    expb_d = nc.dram_tensor("expb", [128, 512], bf, kind="ExternalInput")
    wup_d = nc.dram_tensor("wup", [128, 2 * 1024], f8, kind="ExternalInput")
    wdn_d = nc.dram_tensor("wdn", [128, 8 * 256], f8, kind="ExternalInput")
    dw8_d = nc.dram_tensor("dw8", [128, 8 * 5 * 2 * 128], f8, kind="ExternalInput")
    upb_d = nc.dram_tensor("upb", [128, 8], f32, kind="ExternalInput")
    bnb_d = nc.dram_tensor("bnb", [128, 8], f32, kind="ExternalInput")
    qkb_d = nc.dram_tensor("qkb", [128, 4], f32, kind="ExternalInput")
    vb_d = nc.dram_tensor("vbr", [128, C], f32, kind="ExternalInput")
    dnb_d = nc.dram_tensor("dnb", [128, 2], f32, kind="ExternalInput")
    g2r_d = nc.dram_tensor("g2r", [128, C], f32, kind="ExternalInput")
    b2r_d = nc.dram_tensor("b2r", [128, C], f32, kind="ExternalInput")

    xattn_d = nc.dram_tensor("xattn_s", [TK, C], f32, kind="Internal")
    sc2_d = nc.dram_tensor("sc2_s", [TK, C], bf, kind="Internal")
    out_d = nc.dram_tensor("out", [TO, C], f32, kind="ExternalOutput")

    HEADCOL = [128 * (h % 4) + 64 * (h // 4) for h in range(NH)]
    RIDX = [2 * (h % 4) + (h // 4) for h in range(NH)]

    with tile.TileContext(nc) as tc, ExitStack() as octx:
        consts = octx.enter_context(tc.tile_pool(name="consts", bufs=1))
        persist = octx.enter_context(tc.tile_pool(name="persist", bufs=1))

        eps_sb = consts.tile([128, 1], f32)
        nc.vector.memset(eps_sb[:], 1e-5)
        ident = consts.tile([128, 128], bf)
        make_identity(nc, ident[:])

        wqk_sb = consts.tile([128, 2, 512], f8)
        nc.sync.dma_start(wqk_sb[:], wqk_d[:, :].rearrange("p (j o) -> p j o", j=2))
        wv_sb = consts.tile([128, 2, 256], f8)
        nc.sync.dma_start(wv_sb[:], wv_d[:, :].rearrange("p (j o) -> p j o", j=2))
        wproj_sb = consts.tile([128, 2, 256], f8)
        nc.sync.dma_start(wproj_sb[:], wproj_d[:, :].rearrange("p (j o) -> p j o", j=2))
        expb_sb = consts.tile([128, 512], bf)
        nc.sync.dma_start(expb_sb[:], expb_d[:, :])
        wup_sb = consts.tile([128, 2, 1024], f8)
        nc.sync.dma_start(wup_sb[:], wup_d[:, :].rearrange("p (j o) -> p j o", j=2))
        wdn_sb = consts.tile([128, 8, 256], f8)
        nc.sync.dma_start(wdn_sb[:], wdn_d[:, :].rearrange("p (b o) -> p b o", b=8))
        dw8_sb = consts.tile([128, 8 * 5 * 2 * 128], f8)
        nc.sync.dma_start(dw8_sb[:], dw8_d[:, :])
        upb_sb = consts.tile([128, 8], f32)
        nc.sync.dma_start(upb_sb[:], upb_d[:, :])
        bnb_sb = consts.tile([128, 8], f32)
        nc.sync.dma_start(bnb_sb[:], bnb_d[:, :])
        qkb_sb = consts.tile([128, 4], f32)
        nc.sync.dma_start(qkb_sb[:], qkb_d[:, :])
        vb_sb = consts.tile([128, C], f32)
        nc.sync.dma_start(vb_sb[:], vb_d[:, :])
        dnb_sb = consts.tile([128, 2], f32)
        nc.sync.dma_start(dnb_sb[:], dnb_d[:, :])
        g2r_sb = consts.tile([128, C], f32)
        b2r_sb = consts.tile([128, C], f32)
        if sc2_affine:
            nc.sync.dma_start(g2r_sb[:], g2r_d[:, :])
            nc.sync.dma_start(b2r_sb[:], b2r_d[:, :])

        xn2T = persist.tile([128, 2, TK], f8, tag="xn2T", name="xn2T")

        # ======================= STAGE A =======================
        with ExitStack() as actx:
            xrow_p = actx.enter_context(tc.tile_pool(name="xrow", bufs=18))
            ln_p = actx.enter_context(tc.tile_pool(name="ln", bufs=4))
            mv_p = actx.enter_context(tc.tile_pool(name="mv", bufs=4))
            xnT_p = actx.enter_context(tc.tile_pool(name="xnT", bufs=2))
            qk_p = actx.enter_context(tc.tile_pool(name="qk", bufs=2))
            v_p = actx.enter_context(tc.tile_pool(name="vp", bufs=10))
            at_p = actx.enter_context(tc.tile_pool(name="at", bufs=2))
            small_p = actx.enter_context(tc.tile_pool(name="small", bufs=6))
            proj_p = actx.enter_context(tc.tile_pool(name="proj", bufs=2))
            oT_p = actx.enter_context(tc.tile_pool(name="oTp", bufs=9))
            xat_p = actx.enter_context(tc.tile_pool(name="xatp", bufs=9))
            tail_p = actx.enter_context(tc.tile_pool(name="tail", bufs=3))
            ps_tp = actx.enter_context(tc.tile_pool(name="ps_tp", bufs=2, space="PSUM"))
            ps_mm = actx.enter_context(tc.tile_pool(name="ps_mm", bufs=4, space="PSUM"))
            ps_sc = ps_mm   # qk/v/proj and scores are phase-disjoint per wr
            ps_pat = actx.enter_context(tc.tile_pool(name="ps_pat", bufs=1, space="PSUM"))
            ps_oT = actx.enter_context(tc.tile_pool(name="ps_oT", bufs=1, space="PSUM"))

            def rsqrt_dve(var_ap, tagp):
                """1/sqrt(var+eps) on DVE (arg ~1 for LN of standardized
                data): linear seed + 2 Newton steps.  Avoids the Ln/Exp
                activation-table reloads on the Scalar engine."""
                v = mv_p.tile([128, 8], f32, tag=f"{tagp}v", name=f"{tagp}v")
                nc.vector.tensor_scalar_add(v[:], var_ap, 1e-5)
                y = mv_p.tile([128, 8], f32, tag=f"{tagp}y", name=f"{tagp}y")
                nc.vector.tensor_scalar(out=y[:], in0=v[:], scalar1=-0.5,
                                        scalar2=1.5, op0=ALU.mult, op1=ALU.add)
                for it in range(2):
                    a = mv_p.tile([128, 8], f32, tag=f"{tagp}a{it}", name=f"{tagp}a{it}")
                    nc.vector.tensor_tensor(out=a[:], in0=y[:], in1=y[:], op=ALU.mult)
                    b = mv_p.tile([128, 8], f32, tag=f"{tagp}b{it}", name=f"{tagp}b{it}")
                    nc.vector.tensor_tensor(out=b[:], in0=a[:], in1=v[:], op=ALU.mult)
                    c = mv_p.tile([128, 8], f32, tag=f"{tagp}c{it}", name=f"{tagp}c{it}")
                    nc.vector.tensor_scalar(out=c[:], in0=b[:], scalar1=-0.5,
                                            scalar2=1.5, op0=ALU.mult, op1=ALU.add)
                    y2 = mv_p.tile([128, 8], f32, tag=f"{tagp}y{it}", name=f"{tagp}y{it}")
                    nc.vector.tensor_tensor(out=y2[:], in0=y[:], in1=c[:], op=ALU.mult)
                    y = y2
                return y

            for wr in range(NWR):
                xrows = []
                xnT = xnT_p.tile([128, 2, 1024], f8, tag="xnT", name="xnT")
                mv8 = mv_p.tile([128, 8, 2], f32, tag="mv8", name="mv8")
                for ry in range(8):
                    y = 8 * wr + ry
                    xr = xrow_p.tile([128, C], f32, tag="xrow", name="xrow")
                    nc.sync.dma_start(xr[:], xs_d[y * WI:(y + 1) * WI, :])
                    xrows.append(xr)
                    st = mv_p.tile([128, 6], f32, tag="st", name="st")
                    nc.vector.bn_stats(st[:], xr[:])
                    nc.vector.bn_aggr(mv8[:, ry, :], st[:])
                rstd8 = rsqrt_dve(mv8[:, :, 1], "r1")
                tps = [ps_tp.tile([128, 1024], bf, tag="ps", name="ps")
                       for _ in range(2)]
                for ry in range(8):
                    xnb = ln_p.tile([128, C], bf, tag="xnb", name="xnb")
                    nc.vector.tensor_scalar(
                        out=xnb[:], in0=xrows[ry][:],
                        scalar1=mv8[:, ry, 0:1], scalar2=rstd8[:, ry:ry + 1],
                        op0=ALU.subtract, op1=ALU.mult)
                    for ct in range(2):
                        nc.tensor.transpose(tps[ct][:, 128 * ry:128 * ry + 128],
                                            xnb[:, 128 * ct:128 * ct + 128], ident[:])
                for ct in range(2):
                    # batched scatter: row-major pixel rows -> window-ordered
                    nc.vector.tensor_copy(
                        _sub_ap(xnT[:], 0, 128, ct * 1024,
                                [[8, 8], [64, 16], [1, 8]]),
                        _sub_ap(tps[ct][:], 0, 128, 0,
                                [[128, 8], [8, 16], [1, 8]]))

                # ---- q/k projections (fp8 DR): [oc-tile][128, 1024 tok]
                qkT = [qk_p.tile([128, 1024], bf, tag=f"qkT{m}", name=f"qkT{m}") for m in range(4)]
                for m in range(4):
                    for j in range(2):
                        pqk = ps_mm.tile([128, 512], f32, tag="ps", name="ps")
                        nc.tensor.matmul(pqk[:],
                                         lhsT=wqk_sb[:, :, 128 * m:128 * m + 128],
                                         rhs=xnT[:, :, 512 * j:512 * j + 512],
                                         start=True, stop=True, perf_mode=DR)
                        if qk_bias_nz:
                            nc.vector.tensor_scalar(
                                out=qkT[m][:, 512 * j:512 * j + 512], in0=pqk[:],
                                scalar1=qkb_sb[:, m:m + 1], scalar2=RS,
                                op0=ALU.add, op1=ALU.mult)
                        elif j == 0:
                            nc.vector.tensor_scalar_mul(
                                qkT[m][:, 512 * j:512 * j + 512], pqk[:], RS)
                        else:
                            nc.scalar.mul(qkT[m][:, 512 * j:512 * j + 512],
                                          pqk[:], RS)

                # ---- v per window-pair: [64 tok, 512] (win-major, bf16)
                vsbs = []
                for u in range(8):
                    pv = ps_mm.tile([128, 512], f32, tag="ps", name="ps")
                    for w in range(2):
                        lhsT = xnT[:, :, 64 * (2 * u + w):64 * (2 * u + w) + 64]
                        nc.tensor.matmul(pv[0:64, 256 * w:256 * w + 256],
                                         lhsT=lhsT, rhs=wv_sb[:, :, :],
                                         start=True, stop=True, perf_mode=DR)
                    v2 = v_p.tile([128, 512], bf, tag="v2", name="v2")
                    if v_bias_nz:
                        nc.vector.scalar_tensor_tensor(
                            out=v2[0:64, :], in0=pv[0:64, :], scalar=RS,
                            in1=_sub_ap(vb_sb[:], 0, 64, 0, [[0, 2], [1, C]]),
                            op0=ALU.mult, op1=ALU.add)
                    elif u % 2 == 0:
                        nc.vector.tensor_scalar_mul(v2[0:64, :], pv[0:64, :], RS)
                    else:
                        nc.scalar.mul(v2[0:64, :], pv[0:64, :], RS)
                    vsbs.append(v2)

                # ---- attention per window-pair (bf16), softmax one u ahead
                def emit_scores_softmax(u):
                    pscs = [ps_sc.tile([128, 512], f32, tag="ps", name="ps") for _ in range(4)]
                    for h in range(NH):
                        g, jh = h % 4, h // 4
                        qt_t = qkT[h // 4]
                        kt_t = qkT[2 + h // 4]
                        for w in range(2):
                            tok0 = 64 * (2 * u + w)
                            nc.tensor.matmul(
                                pscs[g][64 * w:64 * w + 64, 64 * jh:64 * jh + 64],
                                lhsT=qt_t[32 * g:32 * g + 32, tok0:tok0 + 64],
                                rhs=kt_t[32 * g:32 * g + 32, tok0:tok0 + 64],
                                start=True, stop=True,
                                tile_position=(32 * g, 64 * w))
                    attn_e = at_p.tile([128, 512], bf, tag="attn_e", name="attn_e")
                    for g in range(4):
                        nc.scalar.activation(attn_e[:, 128 * g:128 * g + 128],
                                             pscs[g][:, 0:128], ACTF.Exp)
                    attn_u = at_p.tile([128, 512], bf, tag="attn_u", name="attn_u")
                    nc.gpsimd.tensor_tensor(out=attn_u[:], in0=attn_e[:],
                                            in1=expb_sb[:], op=ALU.mult)
                    r8 = small_p.tile([128, 8], bf, tag="r8", name="r8")
                    with nc.allow_low_precision(reason="softmax denom bf16"):
                        nc.vector.tensor_reduce(
                            r8[:], attn_u[:].rearrange("p (a k) -> p a k", a=8),
                            axis=AX.X, op=ALU.add)
                    rr8 = small_p.tile([128, 8], bf, tag="rr8", name="rr8")
                    with nc.allow_low_precision(reason="softmax denom bf16"):
                        nc.vector.reciprocal(rr8[:], r8[:])
                    attn_n = at_p.tile([128, 512], bf, tag="attn_n", name="attn_n")
                    nc.vector.tensor_tensor(
                        out=attn_n[:], in0=attn_u[:],
                        in1=_sub_ap(rr8[:], 0, 128, 0, [[1, 8], [0, 64]]),
                        op=ALU.mult)
                    return attn_n

                oTs = []
                pending = emit_scores_softmax(0)
                for u in range(8):
                    attn_n = pending
                    pending = emit_scores_softmax(u + 1) if u + 1 < 8 else None
                    aT = []
                    for g in range(4):
                        pat = ps_pat.tile([128, 256], bf, tag="ps", name="ps")
                        for jh in range(2):
                            nc.tensor.transpose(
                                pat[0:64, 128 * jh:128 * jh + 128],
                                attn_n[:, 128 * g + 64 * jh:128 * g + 64 * jh + 64],
                                ident[:])
                        t = at_p.tile([128, 256], bf, tag=f"aT{g}", name=f"aT{g}")
                        if g < 2:
                            nc.vector.tensor_copy(t[0:64, :], pat[0:64, 0:256])
                        else:
                            nc.scalar.copy(t[0:64, :], pat[0:64, 0:256])
                        aT.append(t)
                    poT = ps_oT.tile([128, 256], f32, tag="ps", name="ps")
                    for h in range(NH):
                        g, jh = h % 4, h // 4
                        for w in range(2):
                            nc.tensor.matmul(
                                poT[32 * g:32 * g + 32,
                                    128 * jh + 64 * w:128 * jh + 64 * w + 64],
                                lhsT=vsbs[u][0:64,
                                             256 * w + 32 * h:256 * w + 32 * h + 32],
                                rhs=aT[g][0:64,
                                          128 * jh + 64 * w:128 * jh + 64 * w + 64],
                                start=True, stop=True, tile_position=(0, 32 * g))
                    oT = oT_p.tile([128, 2, 128], f8, tag="oT", name="oT")
                    nc.vector.tensor_copy(
                        oT[:], poT[:, 0:256].rearrange("p (j n) -> p j n", j=2))
                    oTs.append(oT)

                # ---- proj (fp8 DR): projT [oc-tile][128, 1024] window-order
                projT = [proj_p.tile([128, 1024], bf, tag=f"projT{m}", name=f"projT{m}") for m in range(2)]
                for m in range(2):
                    for jc in range(2):
                        pp = ps_mm.tile([128, 512], f32, tag="ps", name="ps")
                        for uu in range(4):
                            u = 4 * jc + uu
                            nc.tensor.matmul(
                                pp[:, 128 * uu:128 * uu + 128],
                                lhsT=wproj_sb[:, :, 128 * m:128 * m + 128],
                                rhs=oTs[u][:, :, :],
                                start=True, stop=True, perf_mode=DR)
                        for w in range(2):
                            # psum cols (uu, w fixed, iy, ix) -> row-major
                            src = _sub_ap(pp[:], 0, 128, 64 * w,
                                          [[128, 4], [8, 8], [1, 8]])
                            dst = _sub_ap(projT[m][:], 0, 128,
                                          8 * (8 * jc + w), [[16, 4], [128, 8], [1, 8]])
                            if w == 0:
                                nc.vector.tensor_scalar_mul(dst, src, RS)
                            else:
                                nc.scalar.mul(dst, src, RS)

                # ---- tail: per kept row
                if wr == 0:
                    keep = [7]
                elif wr == NWR - 1:
                    keep = [0]
                else:
                    keep = list(range(8))
                mvk = mv_p.tile([128, 8, 2], f32, tag="mvk", name="mvk")
                xat_tiles = {}
                for ry in keep:
                    kk = 8 * wr + ry - 7
                    pfin = ps_pat.tile([128, 256], bf, tag="ps", name="ps")
                    for m in range(2):
                        nc.tensor.transpose(pfin[:, 128 * m:128 * m + 128],
                                            projT[m][:, 128 * ry:128 * ry + 128],
                                            ident[:])
                    xat = xat_p.tile([128, C], f32, tag="xat", name="xat")
                    nc.vector.tensor_tensor(out=xat[:], in0=pfin[:, 0:256],
                                            in1=xrows[ry][:], op=ALU.add)
                    nc.sync.dma_start(xattn_d[kk * WI:(kk + 1) * WI, :], xat[:])
                    st2 = mv_p.tile([128, 6], f32, tag="st2", name="st2")
                    nc.vector.bn_stats(st2[:], xat[:])
                    nc.vector.bn_aggr(mvk[:, ry, :], st2[:])
                    xat_tiles[ry] = xat
                rsk = rsqrt_dve(mvk[:, :, 1], "r2")
                kk0t = 8 * wr + keep[0] - 7
                tq = [ps_tp.tile([128, 1024], bf, tag="ps", name="ps")
                      for _ in range(2)]
                for ji, ry in enumerate(keep):
                    kk = 8 * wr + ry - 7
                    xat = xat_tiles[ry]
                    xn2 = tail_p.tile([128, C], bf, tag="xn2", name="xn2")
                    nc.vector.tensor_scalar(
                        out=xn2[:], in0=xat[:],
                        scalar1=mvk[:, ry, 0:1], scalar2=rsk[:, ry:ry + 1],
                        op0=ALU.subtract, op1=ALU.mult)
                    if sc2_affine:
                        s1 = tail_p.tile([128, C], bf, tag="s1", name="s1")
                        nc.gpsimd.tensor_tensor(out=s1[:], in0=xn2[:], in1=g2r_sb[:],
                                                op=ALU.mult)
                        s2 = tail_p.tile([128, C], bf, tag="s2", name="s2")
                        nc.gpsimd.tensor_tensor(out=s2[:], in0=s1[:], in1=b2r_sb[:],
                                                op=ALU.add)
                        nc.sync.dma_start(sc2_d[kk * WI:(kk + 1) * WI, :], s2[:])
                    else:
                        nc.sync.dma_start(sc2_d[kk * WI:(kk + 1) * WI, :], xn2[:])
                    for ct in range(2):
                        nc.tensor.transpose(tq[ct][:, 128 * ji:128 * ji + 128],
                                            xn2[:, 128 * ct:128 * ct + 128], ident[:])
                for ct in range(2):
                    nk = len(keep)
                    nc.vector.tensor_copy(
                        _sub_ap(xn2T[:], 0, 128, ct * TK + 128 * kk0t,
                                [[1, 128 * nk]]),
                        _sub_ap(tq[ct][:], 0, 128, 0, [[1, 128 * nk]]))

        # ======================= STAGE B =======================
        UW = 130
        USZ = 18 * UW + 1           # +1 pad col for the dummy-tap DR read
        # tap-pair base offsets within a window row block and pair strides
        OFF0 = [0, 2, UW + 1, 2 * UW, 2 * UW + 2]
        DLT = [1, UW - 2, 1, 1, 1]
        with ExitStack() as bctx:
            u_p = bctx.enter_context(tc.tile_pool(name="u_p", bufs=9))
            r_p = bctx.enter_context(tc.tile_pool(name="r_p", bufs=3))
            dnsb_p = bctx.enter_context(tc.tile_pool(name="dnsb", bufs=3))
            fin_p = bctx.enter_context(tc.tile_pool(name="fin", bufs=2))
            psU = bctx.enter_context(tc.tile_pool(name="psU", bufs=2, space="PSUM"))
            psR = bctx.enter_context(tc.tile_pool(name="psR", bufs=2, space="PSUM"))
            psD = bctx.enter_context(tc.tile_pool(name="psD", bufs=2, space="PSUM"))
            psF = bctx.enter_context(tc.tile_pool(name="psF", bufs=2, space="PSUM"))

            for q in range(4):
                kk0 = 16 * q
                Us = []
                for b in range(8):
                    U = u_p.tile([128, USZ], f8, tag="U", name="U")
                    nc.vector.memset(
                        _sub_ap(U[:], 0, 128, 0, [[UW, 18], [129, 2]]), 0.0)
                    # pad col read by the zero-weight dummy tap: 0*NaN = NaN
                    nc.vector.memset(_sub_ap(U[:], 0, 128, 18 * UW, [[1, 1]]), 0.0)
                    for i0 in range(0, 18, 4):
                        nrow = min(4, 18 - i0)
                        pu = psU.tile([128, 512], f32, tag="ps", name="ps")
                        nc.tensor.matmul(
                            pu[:, 0:128 * nrow],
                            lhsT=wup_sb[:, :, 128 * b:128 * b + 128],
                            rhs=xn2T[:, :, (kk0 + i0) * WI:(kk0 + i0 + nrow) * WI],
                            start=True, stop=True, perf_mode=DR)
                        # U carries an extra x64 (psum = 64*up_out); fp8 max
                        # |U| ~ 192 < 240 so the scale is safe
                        udst = _sub_ap(U[:], 0, 128, i0 * UW + 1,
                                       [[UW, nrow], [1, 128]])
                        if upb_nz:
                            nc.scalar.activation(udst, pu[:, 0:128 * nrow],
                                                 ACTF.Relu,
                                                 bias=upb_sb[:, b:b + 1], scale=1.0)
                        elif (i0 // 4 + b) % 2 == 0:
                            nc.vector.tensor_scalar_max(udst, pu[:, 0:128 * nrow],
                                                        0.0)
                        else:
                            nc.scalar.activation(udst, pu[:, 0:128 * nrow],
                                                 ACTF.Relu)
                    Us.append(U)
                for s4 in range(4):
                    jj0 = 4 * s4
                    R8 = r_p.tile([128, 8, 512], f8, tag="R8", name="R8")
                    for b in range(8):
                        pr = psR.tile([128, 512], f32, tag="ps", name="ps")
                        for i in range(5):
                            lhsT = _sub_ap(dw8_sb[:], 0, 128, (b * 5 + i) * 256,
                                           [[128, 2], [1, 128]])
                            rhs = _sub_ap(Us[b][:], 0, 128,
                                          jj0 * UW + OFF0[i],
                                          [[DLT[i], 2], [UW, 4], [1, 128]])
                            nc.tensor.matmul(
                                pr[:, 0:512], lhsT=lhsT, rhs=rhs,
                                start=(i == 0), stop=(i == 4), perf_mode=DR)
                        if bnb_nz:
                            nc.scalar.activation(R8[:, b, :], pr[:], ACTF.Relu,
                                                 bias=bnb_sb[:, b:b + 1], scale=RS)
                        elif b % 2 == 0:
                            nc.vector.tensor_scalar(
                                out=R8[:, b, :], in0=pr[:], scalar1=0.0,
                                scalar2=RS, op0=ALU.max, op1=ALU.mult)
                        else:
                            nc.scalar.activation(R8[:, b, :], pr[:], ACTF.Relu,
                                                 scale=RS)
                    dn = dnsb_p.tile([128, 2, 512], bf, tag="dn", name="dn")
                    for m in range(2):
                        pd = psD.tile([128, 512], f32, tag="ps", name="ps")
                        for bp in range(4):
                            nc.tensor.matmul(
                                pd[:],
                                lhsT=wdn_sb[:, 2 * bp:2 * bp + 2,
                                            128 * m:128 * m + 128],
                                rhs=R8[:, 2 * bp:2 * bp + 2, :],
                                start=(bp == 0), stop=(bp == 3), perf_mode=DR)
                        if dn_bias_nz:
                            nc.vector.tensor_scalar(
                                out=dn[:, m, :], in0=pd[:],
                                scalar1=dnb_sb[:, m:m + 1], scalar2=RS2,
                                op0=ALU.add, op1=ALU.mult)
                        elif m == 0:
                            nc.vector.tensor_scalar_mul(dn[:, m, :], pd[:], RS2)
                        else:
                            nc.scalar.mul(dn[:, m, :], pd[:], RS2)
                    pf = psF.tile([128, 1024], bf, tag="pf", name="pf")
                    for jj in range(4):
                        for m in range(2):
                            nc.tensor.transpose(
                                pf[:, 256 * jj + 128 * m:256 * jj + 128 * m + 128],
                                dn[:, m, 128 * jj:128 * jj + 128], ident[:])
                    j0 = 16 * q + jj0
                    kkf = j0 + 1
                    xa4 = fin_p.tile([128, 4, C], f32, tag="xa4", name="xa4")
                    nc.sync.dma_start(
                        xa4[:],
                        xattn_d[kkf * WI:(kkf + 4) * WI, :]
                        .rearrange("(r p) c -> p r c", p=WI))
                    sc4 = fin_p.tile([128, 4, C], bf, tag="sc4", name="sc4")
                    nc.sync.dma_start(
                        sc4[:],
                        sc2_d[kkf * WI:(kkf + 4) * WI, :]
                        .rearrange("(r p) c -> p r c", p=WI))
                    tmp4 = fin_p.tile([128, 1024], f32, tag="tmp4", name="tmp4")
                    nc.vector.tensor_tensor(
                        out=tmp4[:], in0=pf[:],
                        in1=xa4[:].rearrange("p r c -> p (r c)"), op=ALU.add)
                    out4 = fin_p.tile([128, 1024], f32, tag="out4", name="out4")
                    nc.vector.tensor_tensor(
                        out=out4[:], in0=tmp4[:],
                        in1=sc4[:].rearrange("p r c -> p (r c)"), op=ALU.add)
                    nc.sync.dma_start(
                        out_d[j0 * WI:(j0 + 4) * WI, :]
                        .rearrange("(r p) c -> p r c", p=WI),
                        out4[:].rearrange("p (r c) -> p r c", r=4))

    nc.compile()
    return nc


def _p8(arr, nblk):
    """[nblk*128, N] weight (already scaled) -> [128, nblk*N] fp8 flat with
    col = blk*N + n, i.e. the [p, blk, n] DoubleRow k-subtile layout."""
    n = arr.shape[1]
    return (arr.reshape(nblk, 128, n).transpose(1, 0, 2)
            .reshape(128, nblk * n).astype(F8).copy())


def _prep(g1, b1, qkv_w, qkv_b, rpb_table, rel_idx, proj_w, g2, b2,
          up_w, up_b, dw_w, bn_g, bn_b, down_w, down_b):
    f = np.float32
    g1 = np.asarray(g1, f); b1 = np.asarray(b1, f)
    qkv_w = np.asarray(qkv_w, f); qkv_b = np.asarray(qkv_b, f)
    rpb = np.asarray(rpb_table, f); ridx = np.asarray(rel_idx)
    proj_w = np.asarray(proj_w, f)
    g2 = np.asarray(g2, f); b2 = np.asarray(b2, f)
    up_w = np.asarray(up_w, f); up_b = np.asarray(up_b, f)
    dw_w = np.asarray(dw_w, f); bn_g = np.asarray(bn_g, f)
    bn_b = np.asarray(bn_b, f)
    down_w = np.asarray(down_w, f); down_b = np.asarray(down_b, f)

    sc = HD ** -0.5
    wq = qkv_w[:C] * g1[None, :] * sc
    wk = qkv_w[C:2 * C] * g1[None, :]
    wv = qkv_w[2 * C:] * g1[None, :]
    bq = (qkv_b[:C] + qkv_w[:C] @ b1) * sc
    bk = qkv_b[C:2 * C] + qkv_w[C:2 * C] @ b1
    bv = qkv_b[2 * C:] + qkv_w[2 * C:] @ b1

    wqk8 = _p8(np.concatenate([wq, wk], 0).T * WSC, 2)
    wv8 = _p8(wv.T * WSC, 2)
    wproj8 = _p8(proj_w.T * WSC, 2)

    bias = rpb[np.asarray(ridx).reshape(-1)].reshape(64, 64, NH).transpose(2, 0, 1)
    expb = np.zeros((128, 512), f)
    for h in range(NH):
        cc = 128 * (h % 4) + 64 * (h // 4)
        eb = np.exp(bias[h])
        expb[0:64, cc:cc + 64] = eb
        expb[64:128, cc:cc + 64] = eb
    expb = expb.astype(BF16)

    wup8 = _p8((up_w * g2[None, :]).T * WSC, 2)
    upb = (up_b + up_w @ b2).astype(f)
    bns = bn_g * (1.0 + BN_EPS) ** -0.5
    dww = dw_w.reshape(HID, 9) * bns[:, None] * WSC
    dw8 = np.zeros((128, 8 * 5 * 2 * 128), f)
    pp = np.arange(128)
    for b in range(8):
        for i in range(5):
            for j in range(2):
                t = 2 * i + j
                if t <= 8:
                    dw8[pp, ((b * 5 + i) * 2 + j) * 128 + pp] = dww[128 * b + pp, t]
    dw8 = dw8.astype(F8)
    wdn8 = _p8(down_w.T * WSC, 8)

    def col_n(v, n):
        return np.asarray(v, f).reshape(n, 128).T.copy()

    qkb = col_n(np.concatenate([bq, bk]) * WSC, 4)
    vbr = np.broadcast_to(bv[None, :], (128, C)).astype(f).copy()
    dnb = col_n(down_b, 2)
    g2r = np.broadcast_to(g2[None, :], (128, C)).astype(f).copy()
    b2r = np.broadcast_to(b2[None, :], (128, C)).astype(f).copy()

    flags = (bool(np.any(qkb)), bool(np.any(bv)), bool(np.any(down_b)),
             not (np.allclose(g2, 1.0) and np.allclose(b2, 0.0)),
             bool(np.any(upb)), bool(np.any(bn_b)))

    consts = dict(wqk=wqk8, wv=wv8, wproj=wproj8, expb=expb, wup=wup8,
                  wdn=wdn8, dw8=dw8, upb=col_n(upb * WSC, 8),
                  bnb=col_n(bn_b * WSC, 8),
                  qkb=qkb, vbr=vbr, dnb=col_n(down_b * WSC * WSC, 2),
                  g2r=g2r, b2r=b2r)
    return consts, flags


def kernel(x, H, W, g1, b1, qkv_w, qkv_b, rpb_table, rel_idx, proj_w,
           g2, b2, up_w, up_b, dw_w, bn_g, bn_b, down_w, down_b):
    global LAST_RESULTS
    from concourse.bass_utils import run_bass_kernel_spmd

    x = np.asarray(x, np.float32)
    consts, flags = _prep(g1, b1, qkv_w, qkv_b, rpb_table, rel_idx, proj_w,
                          g2, b2, up_w, up_b, dw_w, bn_g, bn_b, down_w, down_b)
    if flags not in _BUILD_CACHE:
        _BUILD_CACHE[flags] = _build(flags)
    nc = _BUILD_CACHE[flags]

    ximg = x.reshape(B_, HI, WI, C)
    in_maps = []
    for core in range(NCORES):
        b, top = core // 2, (core % 2 == 0)
        r0 = 0 if top else 64
        xs = np.zeros((AROWS, WI, C), np.float32)
        lo, hi = r0 - 8, r0 + 72
        slo, shi = max(lo, 0), min(hi, HI)
        xs[slo - lo:shi - lo] = ximg[b, slo:shi]
        m = {"xs": xs.reshape(TA, C)}
        m.update(consts)
        in_maps.append(m)

    res = run_bass_kernel_spmd(nc, in_maps, core_ids=list(range(NCORES)))
    LAST_RESULTS = res

    out = np.empty((B_, HI, WI, C), np.float32)
    for core in range(NCORES):
        b, top = core // 2, (core % 2 == 0)
        r0 = 0 if top else 64
        out[b, r0:r0 + 64] = res.results[core]["out"].reshape(OROWS, WI, C)
    return out.reshape(B_, HI * WI, C)
